# revision 87
# baseline (speedup 1.0000x reference)
"""EdgeConv 2-layer encoder for Trainium2 (Bass/Tile), edge-direct scheme.

Math (one EdgeConv layer, PyG semantics, aggr='add' over dst):
  msg_e = relu(x[dst_e] @ Wu + x[src_e] @ Wv + ba)   Wu=(A_i-A_j).T, Wv=A_j.T
  agg[n] = sum_{e: dst_e = n} msg_e                  (scatter-add)
  conv[n] = agg[n] @ Wb2 + deg[n] * c0               (BN+linear folded)
  layer1: h = l2norm(relu(conv)); layer2: out = conv

Sharding: edges partitioned by dst across 8 cores; each core owns 49
contiguous 128-node blocks. Within a core, blocks are assigned to
program slots sorted by chunk count so the shared SPMD schedule
(max over cores per slot) wastes only a few % padding.

Host pre-stages per core, in slot order (static graph => static layout):
  xsT  [128, TC*128] bf16  x^T columns gathered by edge src
  sTg  [128, TC*128] fp8   one-hot gather  S^T[m, e] = (dst_loc[e] == m)
  s_in [128, TC*128] fp8   one-hot scatter S[e, m]   (same matrix, transposed)
  xt_own [128, 49*128] bf16  own x^T (slot order) for the node phase
Device, node phase: u'[m,c] = x_own@Wu + ba per block -> persistent SBUF.
Device, per 128-edge chunk (dst confined to one 128-node block):
  msg_psum[e,c] = S^T_ch^T @ u'_blk + xsT_ch^T @ Wv    PE (2 mm, psum accum)
  relu per 4-chunk group: psum -> t bf16               ACT
  aggT[c,m] += t^T @ S_ch  (accumulate over chunks)    PE
Block epilogue ([m, c2] orientation, nodes on partitions):
  conv = aggT^T @ Wb2 + deg x c0; layer1 adds relu + per-partition l2norm.
Layer outputs return to host; host re-gathers for layer 2.
"""

import sys

sys.path.insert(0, "/opt/trn_rl_repo")

import numpy as np

from concourse import bacc, bass, mybir, tile

F32 = mybir.dt.float32
BF16 = mybir.dt.bfloat16
FP8 = mybir.dt.float8e4
BF16_NP = mybir.dt.np(BF16)
FP8_NP = mybir.dt.np(FP8)

C = 128
GRP = 4                   # chunks per relu group (one PSUM bank)
CORES = 8
BPC = 49                  # blocks per core
NPC = BPC * C             # nodes per core 6272
NBT = CORES * BPC
NP = NBT * C              # padded node count 50176
N_NODES = 50000
BN_EPS = 1e-5

LAST = {}                 # timing/info stash for test harness


def build_layer(sched: list[int], apply_norm: bool):
    """One EdgeConv layer program (SPMD, same program all cores).
    sched[j] = chunk count of slot j (shared across cores)."""
    TC = sum(sched)
    maxw = max(sched)
    nc = bacc.Bacc("TRN2", num_swdge_queues=4)

    xsT = nc.declare_dram_parameter("xsT", [C, TC * C], BF16, isOutput=False)
    sTg = nc.declare_dram_parameter("sTg", [C, TC * C], FP8, isOutput=False)
    dstc = nc.declare_dram_parameter("dstc", [C, TC], BF16, isOutput=False)
    ir4 = nc.declare_dram_parameter("ir4", [C, maxw, C], BF16, isOutput=False)
    xt_own = nc.declare_dram_parameter("xt_own", [C, BPC * C], BF16,
                                       isOutput=False)
    wv = nc.declare_dram_parameter("wv", [C, C], BF16, isOutput=False)
    wu = nc.declare_dram_parameter("wu", [C, C], BF16, isOutput=False)
    ba = nc.declare_dram_parameter("ba", [1, C], BF16, isOutput=False)
    ones_r = nc.declare_dram_parameter("ones_r", [1, C], BF16, isOutput=False)
    wb2 = nc.declare_dram_parameter("wb2", [C, C], BF16, isOutput=False)
    c0 = nc.declare_dram_parameter("c0", [1, C], BF16, isOutput=False)
    deg = nc.declare_dram_parameter("deg", [1, BPC * C], BF16, isOutput=False)
    out_t = nc.declare_dram_parameter("out_t", [C, BPC * C], BF16,
                                      isOutput=True)

    with tile.TileContext(nc) as tc:
        with (
            tc.tile_pool(name="constp", bufs=1) as constp,
            tc.tile_pool(name="persist", bufs=1) as persist,
            tc.tile_pool(name="blkin", bufs=4) as blkin,
            tc.tile_pool(name="tpool", bufs=4) as tpool,
            tc.tile_pool(name="sgp", bufs=4) as sgp,
            tc.tile_pool(name="outio", bufs=3) as outio,
            tc.tile_pool(name="msgp", bufs=3, space="PSUM") as msgp,
            tc.tile_pool(name="aggp", bufs=3, space="PSUM") as aggp,
            tc.tile_pool(name="convp", bufs=2, space="PSUM") as convp,
        ):
            wu_sb = constp.tile([C, C], BF16, tag="wu")
            nc.sync.dma_start(out=wu_sb[:], in_=wu[:])
            ba_sb = constp.tile([1, C], BF16, tag="ba")
            nc.sync.dma_start(out=ba_sb[:], in_=ba[:])
            onesr_sb = constp.tile([1, C], BF16, tag="onesr")
            nc.sync.dma_start(out=onesr_sb[:], in_=ones_r[:])
            xo_sb = constp.tile([C, BPC * C], BF16, tag="xo")
            nc.sync.dma_start(out=xo_sb[:, : 4 * C], in_=xt_own[:, : 4 * C])
            wv_sb = constp.tile([C, C], BF16, tag="wv")
            nc.sync.dma_start(out=wv_sb[:], in_=wv[:])
            dstc_sb = constp.tile([C, TC], BF16, tag="dstc")
            nc.sync.dma_start(out=dstc_sb[:], in_=dstc[:])
            ir_sb = constp.tile([C, maxw, C], BF16, tag="ir")
            nc.sync.dma_start(out=ir_sb[:], in_=ir4[:])
            NW = 15
            for w0 in range(4, BPC, NW):
                w1 = min(w0 + NW, BPC)
                nc.sync.dma_start(out=xo_sb[:, w0 * C: w1 * C],
                                  in_=xt_own[:, w0 * C: w1 * C])
            wb2_sb = constp.tile([C, C], BF16, tag="wb2")
            nc.sync.dma_start(out=wb2_sb[:], in_=wb2[:])
            c0_sb = constp.tile([1, C], BF16, tag="c0")
            nc.sync.dma_start(out=c0_sb[:], in_=c0[:])
            deg_sb = constp.tile([1, BPC * C], BF16, tag="deg")
            nc.sync.dma_start(out=deg_sb[:], in_=deg[:])

            # node phase: u'[m, c] = x_own_blk @ Wu + ba, kept in SBUF
            u_sb = persist.tile([C, BPC * C], BF16, tag="u")
            for b in range(BPC):
                ups = msgp.tile([C, GRP * C], F32, tag="msg")
                nc.tensor.matmul(ups[:, :C],
                                 lhsT=xo_sb[:, b * C: (b + 1) * C],
                                 rhs=wu_sb[:], start=True, stop=False)
                nc.tensor.matmul(ups[:, :C], lhsT=onesr_sb[:], rhs=ba_sb[:],
                                 start=False, stop=True)
                nc.vector.tensor_copy(out=u_sb[:, b * C: (b + 1) * C],
                                      in_=ups[:, :C])

            # flat group schedule for 1-group software pipelining of
            # the scatter (PE runs next group's msg mms during relu)
            slot_off = [0]
            for v in sched:
                slot_off.append(slot_off[-1] + v)
            groups = []
            for b in range(BPC):
                nch = sched[b]
                for g0 in range(0, nch, GRP):
                    groups.append((b, g0, min(GRP, nch - g0)))

            blk_tiles = {}
            agg_tiles = {}

            def load_block(b):
                nch, off = sched[b], slot_off[b]
                xs_sb = blkin.tile([C, maxw * C], BF16, tag="xs")
                nc.sync.dma_start(out=xs_sb[:, : nch * C],
                                  in_=xsT[:, off * C: (off + nch) * C])
                st_sb = blkin.tile([C, maxw * C], FP8, tag="st")
                nc.sync.dma_start(out=st_sb[:, : nch * C],
                                  in_=sTg[:, off * C: (off + nch) * C])
                s_blk = sgp.tile([C, maxw, C], BF16, tag="sg",
                                 name=f"s_blk_{b}")
                nc.vector.tensor_tensor(
                    out=s_blk[:, :nch, :],
                    in0=ir_sb[:, :nch, :],
                    in1=dstc_sb[:, off: off + nch]
                        .to_broadcast([C, nch, C]),
                    op=mybir.AluOpType.is_equal)
                blk_tiles[b] = (xs_sb, st_sb, s_blk)

            def emit_msg(item):
                b, g0, gw = item
                xs_sb, st_sb, _ = blk_tiles[b]
                msg = msgp.tile([C, GRP * C], F32, tag="msg")
                for j in range(gw):
                    ch = g0 + j
                    sl = msg[:, j * C: (j + 1) * C]
                    nc.tensor.matmul(sl,
                                     lhsT=st_sb[:, ch * C: (ch + 1) * C],
                                     rhs=u_sb[:, b * C: (b + 1) * C],
                                     start=(j == 0), stop=False)
                    nc.tensor.matmul(sl,
                                     lhsT=xs_sb[:, ch * C: (ch + 1) * C],
                                     rhs=wv_sb[:], start=False,
                                     stop=(j == gw - 1))
                t_g = tpool.tile([C, GRP * C], BF16, tag="t")
                if not apply_norm and (g0 // GRP) % 4 == 3:
                    nc.vector.tensor_scalar(
                        out=t_g[:, : gw * C], in0=msg[:, : gw * C],
                        scalar1=0.0, scalar2=None,
                        op0=mybir.AluOpType.max)
                else:
                    nc.scalar.activation(
                        out=t_g[:, : gw * C], in_=msg[:, : gw * C],
                        func=mybir.ActivationFunctionType.Relu)
                return t_g

            def emit_scatter(item, t_g):
                b, g0, gw = item
                nch = sched[b]
                s_blk = blk_tiles[b][2]
                if b not in agg_tiles:
                    agg_tiles[b] = aggp.tile([C, C], F32, tag="agg",
                                             name=f"aggT_{b}")
                aggT = agg_tiles[b]
                for j in range(gw):
                    ch = g0 + j
                    nc.tensor.matmul(aggT[:],
                                     lhsT=t_g[:, j * C: (j + 1) * C],
                                     rhs=s_blk[:, ch, :],
                                     start=(ch == 0), stop=(ch == nch - 1))

            def emit_cast(b):
                agg_sb = outio.tile([C, C], BF16, tag="aggsb")
                nc.vector.tensor_copy(out=agg_sb[:], in_=agg_tiles.pop(b)[:])
                del blk_tiles[b]
                return agg_sb

            def emit_epilogue(b, agg_sb):
                # conv in [m, c2] orientation: nodes on partitions
                cps = convp.tile([C, C], F32, tag="conv")
                nc.tensor.matmul(cps[:], lhsT=agg_sb[:], rhs=wb2_sb[:],
                                 start=True, stop=False)
                nc.tensor.matmul(cps[:],
                                 lhsT=deg_sb[0:1, b * C: (b + 1) * C],
                                 rhs=c0_sb[:], start=False, stop=True)
                o_sb = outio.tile([C, C], BF16, tag="o")
                if apply_norm:
                    h_sb = outio.tile([C, C], BF16, tag="h")
                    nc.scalar.activation(out=h_sb[:], in_=cps[:],
                                         func=mybir.ActivationFunctionType.Relu)
                    sq_sb = outio.tile([C, C], BF16, tag="sq")
                    nc.vector.tensor_tensor(out=sq_sb[:], in0=h_sb[:],
                                            in1=h_sb[:],
                                            op=mybir.AluOpType.mult)
                    ssq = outio.tile([C, 1], F32, tag="ssq")
                    nc.vector.tensor_reduce(out=ssq[:], in_=sq_sb[:],
                                            axis=mybir.AxisListType.X,
                                            op=mybir.AluOpType.add)
                    nrm = outio.tile([C, 1], F32, tag="nrm")
                    nc.scalar.activation(out=nrm[:], in_=ssq[:],
                                         func=mybir.ActivationFunctionType.Sqrt)
                    nc.vector.tensor_scalar(out=nrm[:], in0=nrm[:],
                                            scalar1=1e-12, scalar2=None,
                                            op0=mybir.AluOpType.max)
                    nc.vector.reciprocal(out=nrm[:], in_=nrm[:])
                    nc.vector.tensor_scalar(out=o_sb[:], in0=h_sb[:],
                                            scalar1=nrm[:], scalar2=None,
                                            op0=mybir.AluOpType.mult)
                else:
                    nc.scalar.activation(out=o_sb[:], in_=cps[:],
                                         func=mybir.ActivationFunctionType.Copy)
                nc.sync.dma_start(out=out_t[:, b * C: (b + 1) * C], in_=o_sb[:])

            from collections import deque
            load_block(0)
            load_block(1)
            pending = deque()  # (item, t_g, s_g) awaiting scatter
            pend_cast = None   # block id awaiting cast
            pend_ep = None     # (b, agg_sb) awaiting conv/norm
            for item in groups:
                b, g0, gw = item
                if g0 == 0 and b + 2 < BPC:
                    load_block(b + 2)
                t_g = emit_msg(item)
                if pend_ep is not None:
                    emit_epilogue(*pend_ep)
                    pend_ep = None
                if len(pending) >= 2:
                    pit, pt = pending.popleft()
                    emit_scatter(pit, pt)
                    pb, pg0, pgw = pit
                    if pg0 + pgw >= sched[pb]:      # block pb finished
                        pend_cast = pb
                if pend_cast is not None:
                    pend_ep = (pend_cast, emit_cast(pend_cast))
                    pend_cast = None
                pending.append((item, t_g))
            if pend_ep is not None:
                emit_epilogue(*pend_ep)
                pend_ep = None
            while pending:
                pit, pt = pending.popleft()
                emit_scatter(pit, pt)
                pb, pg0, pgw = pit
                if pg0 + pgw >= sched[pb]:
                    emit_epilogue(pb, emit_cast(pb))

    nc.compile()
    return nc


# ---------------- host-side data prep ----------------


def prep_layout(dst):
    """Slot-sorted schedule: per core, blocks ordered by descending chunk
    count; sched[j] = max over cores of j-th largest."""
    order_e = np.argsort(dst, kind="stable")
    dst_s = dst[order_e]
    bounds = np.searchsorted(dst_s, np.arange(NBT + 1) * C, side="left")
    counts = (bounds[1:] - bounds[:-1]).reshape(CORES, BPC)
    chunks = -(-counts // C)          # ceil
    block_order = np.argsort(-chunks, axis=1, kind="stable")
    sorted_chunks = -np.sort(-chunks, axis=1)
    sched = sorted_chunks.max(axis=0)
    return sched, block_order, order_e, bounds


def prep_edges(src, dst, sched, block_order, order_e, bounds):
    """Per-core slot-ordered edge arrays, one-hot tables, slot deg."""
    TC = sum(int(x) for x in sched)
    slot_off = np.concatenate([[0], np.cumsum(sched)])
    deg_full = np.bincount(dst, minlength=NP)
    per_core = []
    for k in range(CORES):
        src_perm = np.zeros(TC * C, np.int64)
        dstc_v = np.full(TC * C, 200.0, np.float64)
        deg_slot = np.zeros(BPC * C, np.float64)
        for j in range(BPC):
            blk = int(block_order[k, j])
            g = k * BPC + blk
            e_idx = order_e[bounds[g]: bounds[g + 1]]
            n = len(e_idx)
            base = int(slot_off[j]) * C
            src_perm[base: base + n] = src[e_idx]
            dstc_v[base: base + n] = dst[e_idx] % C
            deg_slot[j * C: (j + 1) * C] = deg_full[k * NPC + blk * C:
                                                    k * NPC + (blk + 1) * C]
        d = dstc_v.reshape(TC, C)
        mr = np.arange(C, dtype=np.float64)
        # gather one-hot sT[m, slot*C + e]; scatter one-hot built on-device
        st_oh = np.ascontiguousarray(
            (d[None, :, :] == mr[:, None, None]).astype(FP8_NP)
        ).reshape(C, TC * C)
        per_core.append({
            "src_perm": src_perm,
            "st_oh": st_oh,
            "dstc": np.ascontiguousarray(d.T.astype(BF16_NP)),
            "deg": np.ascontiguousarray(
                deg_slot.reshape(1, BPC * C).astype(BF16_NP)),
        })
    return per_core


def gather_features(x_bf, per_core, block_order):
    """x_bf [NP, C] bf16 -> per-core (xsT [C, TC*C], xt_own [C, BPC*C])."""
    outs = []
    for k in range(CORES):
        xs = np.ascontiguousarray(x_bf[per_core[k]["src_perm"]].T)
        own = np.empty((BPC, C, C), BF16_NP)
        for j in range(BPC):
            blk = int(block_order[k, j])
            own[j] = x_bf[k * NPC + blk * C: k * NPC + (blk + 1) * C].T
        xo = np.ascontiguousarray(own.transpose(1, 0, 2).reshape(C, BPC * C))
        outs.append((xs, xo))
    return outs


def fold_weights(wa, ba_, g, be, rm, rv, wb, bb, bn_eps=1e-5):
    wa = wa.astype(np.float64)
    A_i, A_j = wa[:, :C], wa[:, C:]
    s = g.astype(np.float64) / np.sqrt(rv.astype(np.float64) + bn_eps)
    wb64 = wb.astype(np.float64)
    wu_m = (A_i - A_j).T
    wv_m = A_j.T
    wb2 = s[:, None] * wb64.T
    c0 = bb.astype(np.float64) + (be.astype(np.float64)
                                  - rm.astype(np.float64) * s) @ wb64.T
    return (wu_m.astype(BF16_NP), wv_m.astype(BF16_NP),
            ba_.astype(BF16_NP).reshape(1, C),
            wb2.astype(BF16_NP), c0.astype(BF16_NP).reshape(1, C))


def _layer_inputs(feat, per_core, wset, maxw):
    wu_m, wv_m, ba_f, wb2, c0 = wset
    onesr = np.ones((1, C), BF16_NP)
    ir = np.tile(np.arange(C, dtype=np.float64), (C, maxw, 1)).astype(BF16_NP)
    in_maps = []
    for k in range(CORES):
        xs, xo = feat[k]
        in_maps.append({
            "xsT": xs, "xt_own": xo,
            "sTg": per_core[k]["st_oh"], "dstc": per_core[k]["dstc"],
            "ir4": ir, "deg": per_core[k]["deg"],
            "wv": wv_m, "wu": wu_m, "ba": ba_f, "wb2": wb2, "c0": c0,
            "ones_r": onesr,
        })
    return in_maps


def assemble_output(results, block_order):
    """Per-core out_t [128(m), BPC*C] slot tiles [m, c2] -> global [NP, C]."""
    h = np.zeros((NP, C), np.float32)
    for k in range(CORES):
        o = np.asarray(results[k]["out_t"]).astype(np.float32)
        o = o.reshape(C, BPC, C).transpose(1, 0, 2)   # [slot, m, c]
        for j in range(BPC):
            blk = int(block_order[k, j])
            h[k * NPC + blk * C: k * NPC + (blk + 1) * C] = o[j]
    return h


# ---------------- device run plumbing ----------------

import os

_NTFF_HOOK = None


def _get_ntff_hook():
    global _NTFF_HOOK
    if _NTFF_HOOK is None:
        sys.path.insert(0, "/root/.axon_site")
        from trn_agent_boot.trn_boot import _ntff_profile_via_ctypes
        _NTFF_HOOK = _ntff_profile_via_ctypes("/opt/axon/libaxon_pjrt.so")
    return _NTFF_HOOK


def _run(nc, in_maps):
    import tempfile
    from concourse import bass2jax
    trace = bool(int(os.environ.get("EDGECONV_TRACE", "0")))
    hook = _get_ntff_hook() if trace else None
    if hook is None:
        results = bass2jax.run_bass_via_pjrt(nc, in_maps, n_cores=CORES)
        LAST.setdefault("exec_ns", []).append(None)
        return results
    neff_dir = tempfile.mkdtemp(prefix="edgeconv_ntff_")
    with hook(neff_dir, [0]):
        results = bass2jax.run_bass_via_pjrt(nc, in_maps, n_cores=CORES)
    exec_ns = None
    try:
        import glob as _glob
        import gauge.profiler
        from concourse._compat import FishPath
        if _glob.glob(os.path.join(neff_dir, "*_body*.ntff")):
            profile = gauge.profiler.Profile(
                profile_path=FishPath(neff_dir), kernel_dev_mode=True,
                profile_on_exit=False, bass_kernel=nc.m,
                offline_processing=True, fname="*_body*")
            pr = profile.to_perfetto(model_index=(0,))
            if pr:
                exec_ns = pr[0].exec_time_ns
                LAST.setdefault("trace_paths", []).append(pr[0].trace_path)
    except Exception as e:  # profiling must never break the kernel
        LAST.setdefault("trace_errors", []).append(repr(e))
    LAST.setdefault("neff_dirs", []).append(neff_dir)
    LAST.setdefault("exec_ns", []).append(exec_ns)
    return results


def kernel(**inputs):
    x = np.asarray(inputs["x"], np.float32)
    edge_index = np.asarray(inputs["edge_index"])
    src = np.asarray(edge_index[0], np.int64)
    dst = np.asarray(edge_index[1], np.int64)

    sched_arr, block_order, order_e, bounds = prep_layout(dst)
    sched = [int(v) for v in sched_arr]
    per_core = prep_edges(src, dst, sched, block_order, order_e, bounds)

    x_pad = np.zeros((NP, C), np.float32)
    x_pad[:N_NODES] = x
    x_bf = x_pad.astype(BF16_NP)

    w1 = fold_weights(np.asarray(inputs["w1a"]), np.asarray(inputs["b1a"]),
                      np.asarray(inputs["g1"]), np.asarray(inputs["be1"]),
                      np.asarray(inputs["rm1"]), np.asarray(inputs["rv1"]),
                      np.asarray(inputs["w1b"]), np.asarray(inputs["b1b"]),
                      BN_EPS)
    w2 = fold_weights(np.asarray(inputs["w2a"]), np.asarray(inputs["b2a"]),
                      np.asarray(inputs["g2"]), np.asarray(inputs["be2"]),
                      np.asarray(inputs["rm2"]), np.asarray(inputs["rv2"]),
                      np.asarray(inputs["w2b"]), np.asarray(inputs["b2b"]),
                      BN_EPS)

    nc1 = build_layer(sched, apply_norm=True)
    r1 = _run(nc1, _layer_inputs(gather_features(x_bf, per_core, block_order),
                                 per_core, w1, max(sched)))
    h = assemble_output(r1, block_order)
    h_bf = h.astype(BF16_NP)

    nc2 = build_layer(sched, apply_norm=False)
    r2 = _run(nc2, _layer_inputs(gather_features(h_bf, per_core, block_order),
                                 per_core, w2, max(sched)))
    out = assemble_output(r2, block_order)

    return np.ascontiguousarray(out[:N_NODES]).astype(np.float32)


# revision 90
# speedup vs baseline: 1.0816x; 1.0816x over previous
"""EdgeConv 2-layer encoder for Trainium2 (Bass/Tile), edge-direct scheme.

Math (one EdgeConv layer, PyG semantics, aggr='add' over dst):
  msg_e = relu(x[dst_e] @ Wu + x[src_e] @ Wv + ba)   Wu=(A_i-A_j).T, Wv=A_j.T
  agg[n] = sum_{e: dst_e = n} msg_e                  (scatter-add)
  conv[n] = agg[n] @ Wb2 + deg[n] * c0               (BN+linear folded)
  layer1: h = l2norm(relu(conv)); layer2: out = conv

Sharding: edges partitioned by dst across 8 cores; each core owns 49
contiguous 128-node blocks. Within a core, blocks are assigned to
program slots sorted by chunk count so the shared SPMD schedule
(max over cores per slot) wastes only a few % padding.

Host pre-stages per core, in slot order (static graph => static layout):
  xsT  [128, TC*128] bf16  x^T columns gathered by edge src
  sTg  [128, TC*128] fp8   one-hot gather  S^T[m, e] = (dst_loc[e] == m)
  s_in [128, TC*128] fp8   one-hot scatter S[e, m]   (same matrix, transposed)
  xt_own [128, 49*128] bf16  own x^T (slot order) for the node phase
Device, node phase: u'[m,c] = x_own@Wu + ba per block -> persistent SBUF.
Device, per 128-edge chunk (dst confined to one 128-node block):
  msg_psum[e,c] = S^T_ch^T @ u'_blk + xsT_ch^T @ Wv    PE (2 mm, psum accum)
  relu per 4-chunk group: psum -> t bf16               ACT
  aggT[c,m] += t^T @ S_ch  (accumulate over chunks)    PE
Block epilogue ([m, c2] orientation, nodes on partitions):
  conv = aggT^T @ Wb2 + deg x c0; layer1 adds relu + per-partition l2norm.
Layer outputs return to host; host re-gathers for layer 2.
"""

import sys

sys.path.insert(0, "/opt/trn_rl_repo")

import numpy as np

from concourse import bacc, bass, mybir, tile

F32 = mybir.dt.float32
BF16 = mybir.dt.bfloat16
FP8 = mybir.dt.float8e4
BF16_NP = mybir.dt.np(BF16)
FP8_NP = mybir.dt.np(FP8)

C = 128
GRP = 4                   # chunks per relu group (one PSUM bank)
CORES = 8
BPC = 49                  # blocks per core
NPC = BPC * C             # nodes per core 6272
NBT = CORES * BPC
NP = NBT * C              # padded node count 50176
N_NODES = 50000
BN_EPS = 1e-5

LAST = {}                 # timing/info stash for test harness


def build_layer(sched: list[int], apply_norm: bool):
    """One EdgeConv layer program (SPMD, same program all cores).
    sched[j] = chunk count of slot j (shared across cores)."""
    TC = sum(sched)
    maxw = max(sched)
    nc = bacc.Bacc("TRN2", num_swdge_queues=4)

    xsT = nc.declare_dram_parameter("xsT", [C, TC * C], BF16, isOutput=False)
    sTg = nc.declare_dram_parameter("sTg", [C, TC * C], FP8, isOutput=False)
    dstc = nc.declare_dram_parameter("dstc", [C, TC], BF16, isOutput=False)
    ir4 = nc.declare_dram_parameter("ir4", [C, maxw, C], BF16, isOutput=False)
    xt_own = nc.declare_dram_parameter("xt_own", [C, BPC * C], BF16,
                                       isOutput=False)
    wv = nc.declare_dram_parameter("wv", [C, C], BF16, isOutput=False)
    wu = nc.declare_dram_parameter("wu", [C, C], BF16, isOutput=False)
    ba = nc.declare_dram_parameter("ba", [1, C], BF16, isOutput=False)
    ones_r = nc.declare_dram_parameter("ones_r", [1, C], BF16, isOutput=False)
    wb2 = nc.declare_dram_parameter("wb2", [C, C], BF16, isOutput=False)
    c0 = nc.declare_dram_parameter("c0", [1, C], BF16, isOutput=False)
    deg = nc.declare_dram_parameter("deg", [1, BPC * C], BF16, isOutput=False)
    out_t = nc.declare_dram_parameter("out_t", [C, BPC * C], BF16,
                                      isOutput=True)

    with tile.TileContext(nc) as tc:
        with (
            tc.tile_pool(name="constp", bufs=1) as constp,
            tc.tile_pool(name="persist", bufs=1) as persist,
            tc.tile_pool(name="blkin", bufs=4) as blkin,
            tc.tile_pool(name="tpool", bufs=4) as tpool,
            tc.tile_pool(name="sgp", bufs=4) as sgp,
            tc.tile_pool(name="outio", bufs=3) as outio,
            tc.tile_pool(name="msgp", bufs=4, space="PSUM") as msgp,
            tc.tile_pool(name="aggp", bufs=3, space="PSUM") as aggp,
            tc.tile_pool(name="convp", bufs=1, space="PSUM") as convp,
        ):
            wu_sb = constp.tile([C, C], BF16, tag="wu")
            nc.sync.dma_start(out=wu_sb[:], in_=wu[:])
            ba_sb = constp.tile([1, C], BF16, tag="ba")
            nc.sync.dma_start(out=ba_sb[:], in_=ba[:])
            onesr_sb = constp.tile([1, C], BF16, tag="onesr")
            nc.sync.dma_start(out=onesr_sb[:], in_=ones_r[:])
            xo_sb = constp.tile([C, BPC * C], BF16, tag="xo")
            nc.sync.dma_start(out=xo_sb[:, : 4 * C], in_=xt_own[:, : 4 * C])
            wv_sb = constp.tile([C, C], BF16, tag="wv")
            nc.sync.dma_start(out=wv_sb[:], in_=wv[:])
            dstc_sb = constp.tile([C, TC], BF16, tag="dstc")
            nc.sync.dma_start(out=dstc_sb[:], in_=dstc[:])
            ir_sb = constp.tile([C, maxw, C], BF16, tag="ir")
            nc.sync.dma_start(out=ir_sb[:], in_=ir4[:])
            NW = 15
            for w0 in range(4, BPC, NW):
                w1 = min(w0 + NW, BPC)
                nc.sync.dma_start(out=xo_sb[:, w0 * C: w1 * C],
                                  in_=xt_own[:, w0 * C: w1 * C])
            wb2_sb = constp.tile([C, C], BF16, tag="wb2")
            nc.sync.dma_start(out=wb2_sb[:], in_=wb2[:])
            c0_sb = constp.tile([1, C], BF16, tag="c0")
            nc.sync.dma_start(out=c0_sb[:], in_=c0[:])
            deg_sb = constp.tile([1, BPC * C], BF16, tag="deg")
            nc.sync.dma_start(out=deg_sb[:], in_=deg[:])

            # node phase: u'[m, c] = x_own_blk @ Wu + ba, kept in SBUF
            u_sb = persist.tile([C, BPC * C], BF16, tag="u")
            for b in range(BPC):
                ups = msgp.tile([C, GRP * C], F32, tag="msg")
                nc.tensor.matmul(ups[:, :C],
                                 lhsT=xo_sb[:, b * C: (b + 1) * C],
                                 rhs=wu_sb[:], start=True, stop=False)
                nc.tensor.matmul(ups[:, :C], lhsT=onesr_sb[:], rhs=ba_sb[:],
                                 start=False, stop=True)
                nc.vector.tensor_copy(out=u_sb[:, b * C: (b + 1) * C],
                                      in_=ups[:, :C])

            # flat group schedule for 1-group software pipelining of
            # the scatter (PE runs next group's msg mms during relu)
            slot_off = [0]
            for v in sched:
                slot_off.append(slot_off[-1] + v)
            groups = []
            for b in range(BPC):
                nch = sched[b]
                for g0 in range(0, nch, GRP):
                    groups.append((b, g0, min(GRP, nch - g0)))

            blk_tiles = {}
            agg_tiles = {}

            def load_block(b):
                nch, off = sched[b], slot_off[b]
                xs_sb = blkin.tile([C, maxw * C], BF16, tag="xs")
                nc.sync.dma_start(out=xs_sb[:, : nch * C],
                                  in_=xsT[:, off * C: (off + nch) * C])
                st_sb = blkin.tile([C, maxw * C], FP8, tag="st")
                nc.sync.dma_start(out=st_sb[:, : nch * C],
                                  in_=sTg[:, off * C: (off + nch) * C])
                s_blk = sgp.tile([C, maxw, C], BF16, tag="sg",
                                 name=f"s_blk_{b}")
                nc.vector.tensor_tensor(
                    out=s_blk[:, :nch, :],
                    in0=ir_sb[:, :nch, :],
                    in1=dstc_sb[:, off: off + nch]
                        .to_broadcast([C, nch, C]),
                    op=mybir.AluOpType.is_equal)
                blk_tiles[b] = (xs_sb, st_sb, s_blk)

            def emit_msg(item):
                b, g0, gw = item
                xs_sb, st_sb, _ = blk_tiles[b]
                msg = msgp.tile([C, GRP * C], F32, tag="msg")
                for j in range(gw):
                    ch = g0 + j
                    sl = msg[:, j * C: (j + 1) * C]
                    nc.tensor.matmul(sl,
                                     lhsT=st_sb[:, ch * C: (ch + 1) * C],
                                     rhs=u_sb[:, b * C: (b + 1) * C],
                                     start=(j == 0), stop=False)
                    nc.tensor.matmul(sl,
                                     lhsT=xs_sb[:, ch * C: (ch + 1) * C],
                                     rhs=wv_sb[:], start=False,
                                     stop=(j == gw - 1))
                t_g = tpool.tile([C, GRP * C], BF16, tag="t")
                emit_msg.ctr += 1
                if not apply_norm and emit_msg.ctr % 5 == 0:
                    nc.vector.tensor_scalar(
                        out=t_g[:, : gw * C], in0=msg[:, : gw * C],
                        scalar1=0.0, scalar2=None,
                        op0=mybir.AluOpType.max)
                else:
                    nc.scalar.activation(
                        out=t_g[:, : gw * C], in_=msg[:, : gw * C],
                        func=mybir.ActivationFunctionType.Relu)
                return t_g

            emit_msg.ctr = 0

            def emit_scatter(item, t_g):
                b, g0, gw = item
                nch = sched[b]
                s_blk = blk_tiles[b][2]
                if b not in agg_tiles:
                    agg_tiles[b] = aggp.tile([C, C], F32, tag="agg",
                                             name=f"aggT_{b}")
                aggT = agg_tiles[b]
                for j in range(gw):
                    ch = g0 + j
                    nc.tensor.matmul(aggT[:],
                                     lhsT=t_g[:, j * C: (j + 1) * C],
                                     rhs=s_blk[:, ch, :],
                                     start=(ch == 0), stop=(ch == nch - 1))

            def emit_cast(b):
                agg_sb = outio.tile([C, C], BF16, tag="aggsb")
                nc.vector.tensor_copy(out=agg_sb[:], in_=agg_tiles.pop(b)[:])
                del blk_tiles[b]
                return agg_sb

            def emit_epilogue(b, agg_sb):
                # conv in [m, c2] orientation: nodes on partitions
                cps = convp.tile([C, C], F32, tag="conv")
                nc.tensor.matmul(cps[:], lhsT=agg_sb[:], rhs=wb2_sb[:],
                                 start=True, stop=False)
                nc.tensor.matmul(cps[:],
                                 lhsT=deg_sb[0:1, b * C: (b + 1) * C],
                                 rhs=c0_sb[:], start=False, stop=True)
                o_sb = outio.tile([C, C], BF16, tag="o")
                if apply_norm:
                    h_sb = outio.tile([C, C], BF16, tag="h")
                    nc.scalar.activation(out=h_sb[:], in_=cps[:],
                                         func=mybir.ActivationFunctionType.Relu)
                    sq_sb = outio.tile([C, C], BF16, tag="sq")
                    nc.vector.tensor_tensor(out=sq_sb[:], in0=h_sb[:],
                                            in1=h_sb[:],
                                            op=mybir.AluOpType.mult)
                    ssq = outio.tile([C, 1], F32, tag="ssq")
                    nc.vector.tensor_reduce(out=ssq[:], in_=sq_sb[:],
                                            axis=mybir.AxisListType.X,
                                            op=mybir.AluOpType.add)
                    nrm = outio.tile([C, 1], F32, tag="nrm")
                    nc.scalar.activation(out=nrm[:], in_=ssq[:],
                                         func=mybir.ActivationFunctionType.Sqrt)
                    nc.vector.tensor_scalar(out=nrm[:], in0=nrm[:],
                                            scalar1=1e-12, scalar2=None,
                                            op0=mybir.AluOpType.max)
                    nc.vector.reciprocal(out=nrm[:], in_=nrm[:])
                    nc.vector.tensor_scalar(out=o_sb[:], in0=h_sb[:],
                                            scalar1=nrm[:], scalar2=None,
                                            op0=mybir.AluOpType.mult)
                else:
                    nc.scalar.activation(out=o_sb[:], in_=cps[:],
                                         func=mybir.ActivationFunctionType.Copy)
                nc.sync.dma_start(out=out_t[:, b * C: (b + 1) * C], in_=o_sb[:])

            from collections import deque
            load_block(0)
            load_block(1)
            pending = deque()  # (item, t_g, s_g) awaiting scatter
            pend_cast = None   # block id awaiting cast
            pend_ep = None     # (b, agg_sb) awaiting conv/norm
            for item in groups:
                b, g0, gw = item
                if g0 == 0 and b + 2 < BPC:
                    load_block(b + 2)
                t_g = emit_msg(item)
                if pend_ep is not None:
                    emit_epilogue(*pend_ep)
                    pend_ep = None
                if len(pending) >= 2:
                    pit, pt = pending.popleft()
                    emit_scatter(pit, pt)
                    pb, pg0, pgw = pit
                    if pg0 + pgw >= sched[pb]:      # block pb finished
                        pend_cast = pb
                if pend_cast is not None:
                    pend_ep = (pend_cast, emit_cast(pend_cast))
                    pend_cast = None
                pending.append((item, t_g))
            if pend_ep is not None:
                emit_epilogue(*pend_ep)
                pend_ep = None
            while pending:
                pit, pt = pending.popleft()
                emit_scatter(pit, pt)
                pb, pg0, pgw = pit
                if pg0 + pgw >= sched[pb]:
                    emit_epilogue(pb, emit_cast(pb))

    nc.compile()
    return nc


# ---------------- host-side data prep ----------------


def prep_layout(dst):
    """Slot-sorted schedule: per core, blocks ordered by descending chunk
    count; sched[j] = max over cores of j-th largest."""
    order_e = np.argsort(dst, kind="stable")
    dst_s = dst[order_e]
    bounds = np.searchsorted(dst_s, np.arange(NBT + 1) * C, side="left")
    counts = (bounds[1:] - bounds[:-1]).reshape(CORES, BPC)
    chunks = -(-counts // C)          # ceil
    block_order = np.argsort(-chunks, axis=1, kind="stable")
    sorted_chunks = -np.sort(-chunks, axis=1)
    sched = sorted_chunks.max(axis=0)
    return sched, block_order, order_e, bounds


def prep_edges(src, dst, sched, block_order, order_e, bounds):
    """Per-core slot-ordered edge arrays, one-hot tables, slot deg."""
    TC = sum(int(x) for x in sched)
    slot_off = np.concatenate([[0], np.cumsum(sched)])
    deg_full = np.bincount(dst, minlength=NP)
    per_core = []
    for k in range(CORES):
        src_perm = np.zeros(TC * C, np.int64)
        dstc_v = np.full(TC * C, 200.0, np.float64)
        deg_slot = np.zeros(BPC * C, np.float64)
        for j in range(BPC):
            blk = int(block_order[k, j])
            g = k * BPC + blk
            e_idx = order_e[bounds[g]: bounds[g + 1]]
            n = len(e_idx)
            base = int(slot_off[j]) * C
            src_perm[base: base + n] = src[e_idx]
            dstc_v[base: base + n] = dst[e_idx] % C
            deg_slot[j * C: (j + 1) * C] = deg_full[k * NPC + blk * C:
                                                    k * NPC + (blk + 1) * C]
        d = dstc_v.reshape(TC, C)
        mr = np.arange(C, dtype=np.float64)
        # gather one-hot sT[m, slot*C + e]; scatter one-hot built on-device
        st_oh = np.ascontiguousarray(
            (d[None, :, :] == mr[:, None, None]).astype(FP8_NP)
        ).reshape(C, TC * C)
        per_core.append({
            "src_perm": src_perm,
            "st_oh": st_oh,
            "dstc": np.ascontiguousarray(d.T.astype(BF16_NP)),
            "deg": np.ascontiguousarray(
                deg_slot.reshape(1, BPC * C).astype(BF16_NP)),
        })
    return per_core


def gather_features(x_bf, per_core, block_order):
    """x_bf [NP, C] bf16 -> per-core (xsT [C, TC*C], xt_own [C, BPC*C])."""
    outs = []
    for k in range(CORES):
        xs = np.ascontiguousarray(x_bf[per_core[k]["src_perm"]].T)
        own = np.empty((BPC, C, C), BF16_NP)
        for j in range(BPC):
            blk = int(block_order[k, j])
            own[j] = x_bf[k * NPC + blk * C: k * NPC + (blk + 1) * C].T
        xo = np.ascontiguousarray(own.transpose(1, 0, 2).reshape(C, BPC * C))
        outs.append((xs, xo))
    return outs


def fold_weights(wa, ba_, g, be, rm, rv, wb, bb, bn_eps=1e-5):
    wa = wa.astype(np.float64)
    A_i, A_j = wa[:, :C], wa[:, C:]
    s = g.astype(np.float64) / np.sqrt(rv.astype(np.float64) + bn_eps)
    wb64 = wb.astype(np.float64)
    wu_m = (A_i - A_j).T
    wv_m = A_j.T
    wb2 = s[:, None] * wb64.T
    c0 = bb.astype(np.float64) + (be.astype(np.float64)
                                  - rm.astype(np.float64) * s) @ wb64.T
    return (wu_m.astype(BF16_NP), wv_m.astype(BF16_NP),
            ba_.astype(BF16_NP).reshape(1, C),
            wb2.astype(BF16_NP), c0.astype(BF16_NP).reshape(1, C))


def _layer_inputs(feat, per_core, wset, maxw):
    wu_m, wv_m, ba_f, wb2, c0 = wset
    onesr = np.ones((1, C), BF16_NP)
    ir = np.tile(np.arange(C, dtype=np.float64), (C, maxw, 1)).astype(BF16_NP)
    in_maps = []
    for k in range(CORES):
        xs, xo = feat[k]
        in_maps.append({
            "xsT": xs, "xt_own": xo,
            "sTg": per_core[k]["st_oh"], "dstc": per_core[k]["dstc"],
            "ir4": ir, "deg": per_core[k]["deg"],
            "wv": wv_m, "wu": wu_m, "ba": ba_f, "wb2": wb2, "c0": c0,
            "ones_r": onesr,
        })
    return in_maps


def assemble_output(results, block_order):
    """Per-core out_t [128(m), BPC*C] slot tiles [m, c2] -> global [NP, C]."""
    h = np.zeros((NP, C), np.float32)
    for k in range(CORES):
        o = np.asarray(results[k]["out_t"]).astype(np.float32)
        o = o.reshape(C, BPC, C).transpose(1, 0, 2)   # [slot, m, c]
        for j in range(BPC):
            blk = int(block_order[k, j])
            h[k * NPC + blk * C: k * NPC + (blk + 1) * C] = o[j]
    return h


# ---------------- device run plumbing ----------------

import os

_NTFF_HOOK = None


def _get_ntff_hook():
    global _NTFF_HOOK
    if _NTFF_HOOK is None:
        sys.path.insert(0, "/root/.axon_site")
        from trn_agent_boot.trn_boot import _ntff_profile_via_ctypes
        _NTFF_HOOK = _ntff_profile_via_ctypes("/opt/axon/libaxon_pjrt.so")
    return _NTFF_HOOK


def _run(nc, in_maps):
    import tempfile
    from concourse import bass2jax
    trace = bool(int(os.environ.get("EDGECONV_TRACE", "0")))
    hook = _get_ntff_hook() if trace else None
    if hook is None:
        results = bass2jax.run_bass_via_pjrt(nc, in_maps, n_cores=CORES)
        LAST.setdefault("exec_ns", []).append(None)
        return results
    neff_dir = tempfile.mkdtemp(prefix="edgeconv_ntff_")
    with hook(neff_dir, [0]):
        results = bass2jax.run_bass_via_pjrt(nc, in_maps, n_cores=CORES)
    exec_ns = None
    try:
        import glob as _glob
        import gauge.profiler
        from concourse._compat import FishPath
        if _glob.glob(os.path.join(neff_dir, "*_body*.ntff")):
            profile = gauge.profiler.Profile(
                profile_path=FishPath(neff_dir), kernel_dev_mode=True,
                profile_on_exit=False, bass_kernel=nc.m,
                offline_processing=True, fname="*_body*")
            pr = profile.to_perfetto(model_index=(0,))
            if pr:
                exec_ns = pr[0].exec_time_ns
                LAST.setdefault("trace_paths", []).append(pr[0].trace_path)
    except Exception as e:  # profiling must never break the kernel
        LAST.setdefault("trace_errors", []).append(repr(e))
    LAST.setdefault("neff_dirs", []).append(neff_dir)
    LAST.setdefault("exec_ns", []).append(exec_ns)
    return results


def kernel(**inputs):
    x = np.asarray(inputs["x"], np.float32)
    edge_index = np.asarray(inputs["edge_index"])
    src = np.asarray(edge_index[0], np.int64)
    dst = np.asarray(edge_index[1], np.int64)

    sched_arr, block_order, order_e, bounds = prep_layout(dst)
    sched = [int(v) for v in sched_arr]
    per_core = prep_edges(src, dst, sched, block_order, order_e, bounds)

    x_pad = np.zeros((NP, C), np.float32)
    x_pad[:N_NODES] = x
    x_bf = x_pad.astype(BF16_NP)

    w1 = fold_weights(np.asarray(inputs["w1a"]), np.asarray(inputs["b1a"]),
                      np.asarray(inputs["g1"]), np.asarray(inputs["be1"]),
                      np.asarray(inputs["rm1"]), np.asarray(inputs["rv1"]),
                      np.asarray(inputs["w1b"]), np.asarray(inputs["b1b"]),
                      BN_EPS)
    w2 = fold_weights(np.asarray(inputs["w2a"]), np.asarray(inputs["b2a"]),
                      np.asarray(inputs["g2"]), np.asarray(inputs["be2"]),
                      np.asarray(inputs["rm2"]), np.asarray(inputs["rv2"]),
                      np.asarray(inputs["w2b"]), np.asarray(inputs["b2b"]),
                      BN_EPS)

    nc1 = build_layer(sched, apply_norm=True)
    r1 = _run(nc1, _layer_inputs(gather_features(x_bf, per_core, block_order),
                                 per_core, w1, max(sched)))
    h = assemble_output(r1, block_order)
    h_bf = h.astype(BF16_NP)

    nc2 = build_layer(sched, apply_norm=False)
    r2 = _run(nc2, _layer_inputs(gather_features(h_bf, per_core, block_order),
                                 per_core, w2, max(sched)))
    out = assemble_output(r2, block_order)

    return np.ascontiguousarray(out[:N_NODES]).astype(np.float32)


# revision 91
# speedup vs baseline: 1.0893x; 1.0070x over previous
"""EdgeConv 2-layer encoder for Trainium2 (Bass/Tile), edge-direct scheme.

Math (one EdgeConv layer, PyG semantics, aggr='add' over dst):
  msg_e = relu(x[dst_e] @ Wu + x[src_e] @ Wv + ba)   Wu=(A_i-A_j).T, Wv=A_j.T
  agg[n] = sum_{e: dst_e = n} msg_e                  (scatter-add)
  conv[n] = agg[n] @ Wb2 + deg[n] * c0               (BN+linear folded)
  layer1: h = l2norm(relu(conv)); layer2: out = conv

Sharding: edges partitioned by dst across 8 cores; each core owns 49
contiguous 128-node blocks. Within a core, blocks are assigned to
program slots sorted by chunk count so the shared SPMD schedule
(max over cores per slot) wastes only a few % padding.

Host pre-stages per core, in slot order (static graph => static layout):
  xsT  [128, TC*128] bf16  x^T columns gathered by edge src
  sTg  [128, TC*128] fp8   one-hot gather  S^T[m, e] = (dst_loc[e] == m)
  s_in [128, TC*128] fp8   one-hot scatter S[e, m]   (same matrix, transposed)
  xt_own [128, 49*128] bf16  own x^T (slot order) for the node phase
Device, node phase: u'[m,c] = x_own@Wu + ba per block -> persistent SBUF.
Device, per 128-edge chunk (dst confined to one 128-node block):
  msg_psum[e,c] = S^T_ch^T @ u'_blk + xsT_ch^T @ Wv    PE (2 mm, psum accum)
  relu per 4-chunk group: psum -> t bf16               ACT
  aggT[c,m] += t^T @ S_ch  (accumulate over chunks)    PE
Block epilogue ([m, c2] orientation, nodes on partitions):
  conv = aggT^T @ Wb2 + deg x c0; layer1 adds relu + per-partition l2norm.
Layer outputs return to host; host re-gathers for layer 2.
"""

import sys

sys.path.insert(0, "/opt/trn_rl_repo")

import numpy as np

from concourse import bacc, bass, mybir, tile

F32 = mybir.dt.float32
BF16 = mybir.dt.bfloat16
FP8 = mybir.dt.float8e4
BF16_NP = mybir.dt.np(BF16)
FP8_NP = mybir.dt.np(FP8)

C = 128
GRP = 4                   # chunks per relu group (one PSUM bank)
CORES = 8
BPC = 49                  # blocks per core
NPC = BPC * C             # nodes per core 6272
NBT = CORES * BPC
NP = NBT * C              # padded node count 50176
N_NODES = 50000
BN_EPS = 1e-5

LAST = {}                 # timing/info stash for test harness


def build_layer(sched: list[int], apply_norm: bool):
    """One EdgeConv layer program (SPMD, same program all cores).
    sched[j] = chunk count of slot j (shared across cores)."""
    TC = sum(sched)
    maxw = max(sched)
    nc = bacc.Bacc("TRN2", num_swdge_queues=4)

    xsT = nc.declare_dram_parameter("xsT", [C, TC * C], BF16, isOutput=False)
    sTg = nc.declare_dram_parameter("sTg", [C, TC * C], FP8, isOutput=False)
    dstc = nc.declare_dram_parameter("dstc", [C, TC], BF16, isOutput=False)
    ir4 = nc.declare_dram_parameter("ir4", [C, maxw, C], BF16, isOutput=False)
    xt_own = nc.declare_dram_parameter("xt_own", [C, BPC * C], BF16,
                                       isOutput=False)
    wv = nc.declare_dram_parameter("wv", [C, C], BF16, isOutput=False)
    wu = nc.declare_dram_parameter("wu", [C, C], BF16, isOutput=False)
    ba = nc.declare_dram_parameter("ba", [1, C], BF16, isOutput=False)
    ones_r = nc.declare_dram_parameter("ones_r", [1, C], BF16, isOutput=False)
    wb2 = nc.declare_dram_parameter("wb2", [C, C], BF16, isOutput=False)
    c0 = nc.declare_dram_parameter("c0", [1, C], BF16, isOutput=False)
    deg = nc.declare_dram_parameter("deg", [1, BPC * C], BF16, isOutput=False)
    out_t = nc.declare_dram_parameter("out_t", [C, BPC * C], BF16,
                                      isOutput=True)

    with tile.TileContext(nc) as tc:
        with (
            tc.tile_pool(name="constp", bufs=1) as constp,
            tc.tile_pool(name="persist", bufs=1) as persist,
            tc.tile_pool(name="blkin", bufs=4) as blkin,
            tc.tile_pool(name="tpool", bufs=4) as tpool,
            tc.tile_pool(name="sgp", bufs=4) as sgp,
            tc.tile_pool(name="outio", bufs=3) as outio,
            tc.tile_pool(name="msgp", bufs=4, space="PSUM") as msgp,
            tc.tile_pool(name="aggp", bufs=3, space="PSUM") as aggp,
            tc.tile_pool(name="convp", bufs=1, space="PSUM") as convp,
        ):
            wu_sb = constp.tile([C, C], BF16, tag="wu")
            nc.sync.dma_start(out=wu_sb[:], in_=wu[:])
            ba_sb = constp.tile([1, C], BF16, tag="ba")
            nc.sync.dma_start(out=ba_sb[:], in_=ba[:])
            onesr_sb = constp.tile([1, C], BF16, tag="onesr")
            nc.sync.dma_start(out=onesr_sb[:], in_=ones_r[:])
            xo_sb = constp.tile([C, BPC * C], BF16, tag="xo")
            nc.sync.dma_start(out=xo_sb[:, : 4 * C], in_=xt_own[:, : 4 * C])
            wv_sb = constp.tile([C, C], BF16, tag="wv")
            nc.sync.dma_start(out=wv_sb[:], in_=wv[:])
            NW = 15
            for w0 in range(4, BPC, NW):
                w1 = min(w0 + NW, BPC)
                nc.sync.dma_start(out=xo_sb[:, w0 * C: w1 * C],
                                  in_=xt_own[:, w0 * C: w1 * C])
            dstc_sb = constp.tile([C, TC], BF16, tag="dstc")
            nc.sync.dma_start(out=dstc_sb[:], in_=dstc[:])
            ir_sb = constp.tile([C, maxw, C], BF16, tag="ir")
            nc.sync.dma_start(out=ir_sb[:], in_=ir4[:])
            wb2_sb = constp.tile([C, C], BF16, tag="wb2")
            nc.sync.dma_start(out=wb2_sb[:], in_=wb2[:])
            c0_sb = constp.tile([1, C], BF16, tag="c0")
            nc.sync.dma_start(out=c0_sb[:], in_=c0[:])
            deg_sb = constp.tile([1, BPC * C], BF16, tag="deg")
            nc.sync.dma_start(out=deg_sb[:], in_=deg[:])

            # node phase: u'[m, c] = x_own_blk @ Wu + ba, kept in SBUF
            u_sb = persist.tile([C, BPC * C], BF16, tag="u")
            for b in range(BPC):
                ups = msgp.tile([C, GRP * C], F32, tag="msg")
                nc.tensor.matmul(ups[:, :C],
                                 lhsT=xo_sb[:, b * C: (b + 1) * C],
                                 rhs=wu_sb[:], start=True, stop=False)
                nc.tensor.matmul(ups[:, :C], lhsT=onesr_sb[:], rhs=ba_sb[:],
                                 start=False, stop=True)
                nc.vector.tensor_copy(out=u_sb[:, b * C: (b + 1) * C],
                                      in_=ups[:, :C])

            # flat group schedule for 1-group software pipelining of
            # the scatter (PE runs next group's msg mms during relu)
            slot_off = [0]
            for v in sched:
                slot_off.append(slot_off[-1] + v)
            groups = []
            for b in range(BPC):
                nch = sched[b]
                for g0 in range(0, nch, GRP):
                    groups.append((b, g0, min(GRP, nch - g0)))

            blk_tiles = {}
            agg_tiles = {}

            def load_block(b):
                nch, off = sched[b], slot_off[b]
                xs_sb = blkin.tile([C, maxw * C], BF16, tag="xs")
                nc.sync.dma_start(out=xs_sb[:, : nch * C],
                                  in_=xsT[:, off * C: (off + nch) * C])
                st_sb = blkin.tile([C, maxw * C], FP8, tag="st")
                nc.sync.dma_start(out=st_sb[:, : nch * C],
                                  in_=sTg[:, off * C: (off + nch) * C])
                s_blk = sgp.tile([C, maxw, C], BF16, tag="sg",
                                 name=f"s_blk_{b}")
                nc.vector.tensor_tensor(
                    out=s_blk[:, :nch, :],
                    in0=ir_sb[:, :nch, :],
                    in1=dstc_sb[:, off: off + nch]
                        .to_broadcast([C, nch, C]),
                    op=mybir.AluOpType.is_equal)
                blk_tiles[b] = (xs_sb, st_sb, s_blk)

            def emit_msg(item):
                b, g0, gw = item
                xs_sb, st_sb, _ = blk_tiles[b]
                msg = msgp.tile([C, GRP * C], F32, tag="msg")
                for j in range(gw):
                    ch = g0 + j
                    sl = msg[:, j * C: (j + 1) * C]
                    nc.tensor.matmul(sl,
                                     lhsT=st_sb[:, ch * C: (ch + 1) * C],
                                     rhs=u_sb[:, b * C: (b + 1) * C],
                                     start=(j == 0), stop=False)
                    nc.tensor.matmul(sl,
                                     lhsT=xs_sb[:, ch * C: (ch + 1) * C],
                                     rhs=wv_sb[:], start=False,
                                     stop=(j == gw - 1))
                t_g = tpool.tile([C, GRP * C], BF16, tag="t")
                emit_msg.ctr += 1
                if not apply_norm and emit_msg.ctr % 5 == 0:
                    nc.vector.tensor_scalar(
                        out=t_g[:, : gw * C], in0=msg[:, : gw * C],
                        scalar1=0.0, scalar2=None,
                        op0=mybir.AluOpType.max)
                else:
                    nc.scalar.activation(
                        out=t_g[:, : gw * C], in_=msg[:, : gw * C],
                        func=mybir.ActivationFunctionType.Relu)
                return t_g

            emit_msg.ctr = 0

            def emit_scatter(item, t_g):
                b, g0, gw = item
                nch = sched[b]
                s_blk = blk_tiles[b][2]
                if b not in agg_tiles:
                    agg_tiles[b] = aggp.tile([C, C], F32, tag="agg",
                                             name=f"aggT_{b}")
                aggT = agg_tiles[b]
                for j in range(gw):
                    ch = g0 + j
                    nc.tensor.matmul(aggT[:],
                                     lhsT=t_g[:, j * C: (j + 1) * C],
                                     rhs=s_blk[:, ch, :],
                                     start=(ch == 0), stop=(ch == nch - 1))

            def emit_cast(b):
                agg_sb = outio.tile([C, C], BF16, tag="aggsb")
                nc.vector.tensor_copy(out=agg_sb[:], in_=agg_tiles.pop(b)[:])
                del blk_tiles[b]
                return agg_sb

            def emit_epilogue(b, agg_sb):
                # conv in [m, c2] orientation: nodes on partitions
                cps = convp.tile([C, C], F32, tag="conv")
                nc.tensor.matmul(cps[:], lhsT=agg_sb[:], rhs=wb2_sb[:],
                                 start=True, stop=False)
                nc.tensor.matmul(cps[:],
                                 lhsT=deg_sb[0:1, b * C: (b + 1) * C],
                                 rhs=c0_sb[:], start=False, stop=True)
                o_sb = outio.tile([C, C], BF16, tag="o")
                if apply_norm:
                    h_sb = outio.tile([C, C], BF16, tag="h")
                    nc.scalar.activation(out=h_sb[:], in_=cps[:],
                                         func=mybir.ActivationFunctionType.Relu)
                    sq_sb = outio.tile([C, C], BF16, tag="sq")
                    nc.vector.tensor_tensor(out=sq_sb[:], in0=h_sb[:],
                                            in1=h_sb[:],
                                            op=mybir.AluOpType.mult)
                    ssq = outio.tile([C, 1], F32, tag="ssq")
                    nc.vector.tensor_reduce(out=ssq[:], in_=sq_sb[:],
                                            axis=mybir.AxisListType.X,
                                            op=mybir.AluOpType.add)
                    nrm = outio.tile([C, 1], F32, tag="nrm")
                    nc.scalar.activation(out=nrm[:], in_=ssq[:],
                                         func=mybir.ActivationFunctionType.Sqrt)
                    nc.vector.tensor_scalar(out=nrm[:], in0=nrm[:],
                                            scalar1=1e-12, scalar2=None,
                                            op0=mybir.AluOpType.max)
                    nc.vector.reciprocal(out=nrm[:], in_=nrm[:])
                    nc.vector.tensor_scalar(out=o_sb[:], in0=h_sb[:],
                                            scalar1=nrm[:], scalar2=None,
                                            op0=mybir.AluOpType.mult)
                else:
                    nc.scalar.activation(out=o_sb[:], in_=cps[:],
                                         func=mybir.ActivationFunctionType.Copy)
                nc.sync.dma_start(out=out_t[:, b * C: (b + 1) * C], in_=o_sb[:])

            from collections import deque
            load_block(0)
            load_block(1)
            pending = deque()  # (item, t_g, s_g) awaiting scatter
            pend_cast = None   # block id awaiting cast
            pend_ep = None     # (b, agg_sb) awaiting conv/norm
            for item in groups:
                b, g0, gw = item
                if g0 == 0 and b + 2 < BPC:
                    load_block(b + 2)
                t_g = emit_msg(item)
                if pend_ep is not None:
                    emit_epilogue(*pend_ep)
                    pend_ep = None
                if len(pending) >= 2:
                    pit, pt = pending.popleft()
                    emit_scatter(pit, pt)
                    pb, pg0, pgw = pit
                    if pg0 + pgw >= sched[pb]:      # block pb finished
                        pend_cast = pb
                if pend_cast is not None:
                    pend_ep = (pend_cast, emit_cast(pend_cast))
                    pend_cast = None
                pending.append((item, t_g))
            if pend_ep is not None:
                emit_epilogue(*pend_ep)
                pend_ep = None
            while pending:
                pit, pt = pending.popleft()
                emit_scatter(pit, pt)
                pb, pg0, pgw = pit
                if pg0 + pgw >= sched[pb]:
                    emit_epilogue(pb, emit_cast(pb))

    nc.compile()
    return nc


# ---------------- host-side data prep ----------------


def prep_layout(dst):
    """Slot-sorted schedule: per core, blocks ordered by descending chunk
    count; sched[j] = max over cores of j-th largest."""
    order_e = np.argsort(dst, kind="stable")
    dst_s = dst[order_e]
    bounds = np.searchsorted(dst_s, np.arange(NBT + 1) * C, side="left")
    counts = (bounds[1:] - bounds[:-1]).reshape(CORES, BPC)
    chunks = -(-counts // C)          # ceil
    block_order = np.argsort(-chunks, axis=1, kind="stable")
    sorted_chunks = -np.sort(-chunks, axis=1)
    sched = sorted_chunks.max(axis=0)
    return sched, block_order, order_e, bounds


def prep_edges(src, dst, sched, block_order, order_e, bounds):
    """Per-core slot-ordered edge arrays, one-hot tables, slot deg."""
    TC = sum(int(x) for x in sched)
    slot_off = np.concatenate([[0], np.cumsum(sched)])
    deg_full = np.bincount(dst, minlength=NP)
    per_core = []
    for k in range(CORES):
        src_perm = np.zeros(TC * C, np.int64)
        dstc_v = np.full(TC * C, 200.0, np.float64)
        deg_slot = np.zeros(BPC * C, np.float64)
        for j in range(BPC):
            blk = int(block_order[k, j])
            g = k * BPC + blk
            e_idx = order_e[bounds[g]: bounds[g + 1]]
            n = len(e_idx)
            base = int(slot_off[j]) * C
            src_perm[base: base + n] = src[e_idx]
            dstc_v[base: base + n] = dst[e_idx] % C
            deg_slot[j * C: (j + 1) * C] = deg_full[k * NPC + blk * C:
                                                    k * NPC + (blk + 1) * C]
        d = dstc_v.reshape(TC, C)
        mr = np.arange(C, dtype=np.float64)
        # gather one-hot sT[m, slot*C + e]; scatter one-hot built on-device
        st_oh = np.ascontiguousarray(
            (d[None, :, :] == mr[:, None, None]).astype(FP8_NP)
        ).reshape(C, TC * C)
        per_core.append({
            "src_perm": src_perm,
            "st_oh": st_oh,
            "dstc": np.ascontiguousarray(d.T.astype(BF16_NP)),
            "deg": np.ascontiguousarray(
                deg_slot.reshape(1, BPC * C).astype(BF16_NP)),
        })
    return per_core


def gather_features(x_bf, per_core, block_order):
    """x_bf [NP, C] bf16 -> per-core (xsT [C, TC*C], xt_own [C, BPC*C])."""
    outs = []
    for k in range(CORES):
        xs = np.ascontiguousarray(x_bf[per_core[k]["src_perm"]].T)
        own = np.empty((BPC, C, C), BF16_NP)
        for j in range(BPC):
            blk = int(block_order[k, j])
            own[j] = x_bf[k * NPC + blk * C: k * NPC + (blk + 1) * C].T
        xo = np.ascontiguousarray(own.transpose(1, 0, 2).reshape(C, BPC * C))
        outs.append((xs, xo))
    return outs


def fold_weights(wa, ba_, g, be, rm, rv, wb, bb, bn_eps=1e-5):
    wa = wa.astype(np.float64)
    A_i, A_j = wa[:, :C], wa[:, C:]
    s = g.astype(np.float64) / np.sqrt(rv.astype(np.float64) + bn_eps)
    wb64 = wb.astype(np.float64)
    wu_m = (A_i - A_j).T
    wv_m = A_j.T
    wb2 = s[:, None] * wb64.T
    c0 = bb.astype(np.float64) + (be.astype(np.float64)
                                  - rm.astype(np.float64) * s) @ wb64.T
    return (wu_m.astype(BF16_NP), wv_m.astype(BF16_NP),
            ba_.astype(BF16_NP).reshape(1, C),
            wb2.astype(BF16_NP), c0.astype(BF16_NP).reshape(1, C))


def _layer_inputs(feat, per_core, wset, maxw):
    wu_m, wv_m, ba_f, wb2, c0 = wset
    onesr = np.ones((1, C), BF16_NP)
    ir = np.tile(np.arange(C, dtype=np.float64), (C, maxw, 1)).astype(BF16_NP)
    in_maps = []
    for k in range(CORES):
        xs, xo = feat[k]
        in_maps.append({
            "xsT": xs, "xt_own": xo,
            "sTg": per_core[k]["st_oh"], "dstc": per_core[k]["dstc"],
            "ir4": ir, "deg": per_core[k]["deg"],
            "wv": wv_m, "wu": wu_m, "ba": ba_f, "wb2": wb2, "c0": c0,
            "ones_r": onesr,
        })
    return in_maps


def assemble_output(results, block_order):
    """Per-core out_t [128(m), BPC*C] slot tiles [m, c2] -> global [NP, C]."""
    h = np.zeros((NP, C), np.float32)
    for k in range(CORES):
        o = np.asarray(results[k]["out_t"]).astype(np.float32)
        o = o.reshape(C, BPC, C).transpose(1, 0, 2)   # [slot, m, c]
        for j in range(BPC):
            blk = int(block_order[k, j])
            h[k * NPC + blk * C: k * NPC + (blk + 1) * C] = o[j]
    return h


# ---------------- device run plumbing ----------------

import os

_NTFF_HOOK = None


def _get_ntff_hook():
    global _NTFF_HOOK
    if _NTFF_HOOK is None:
        sys.path.insert(0, "/root/.axon_site")
        from trn_agent_boot.trn_boot import _ntff_profile_via_ctypes
        _NTFF_HOOK = _ntff_profile_via_ctypes("/opt/axon/libaxon_pjrt.so")
    return _NTFF_HOOK


def _run(nc, in_maps):
    import tempfile
    from concourse import bass2jax
    trace = bool(int(os.environ.get("EDGECONV_TRACE", "0")))
    hook = _get_ntff_hook() if trace else None
    if hook is None:
        results = bass2jax.run_bass_via_pjrt(nc, in_maps, n_cores=CORES)
        LAST.setdefault("exec_ns", []).append(None)
        return results
    neff_dir = tempfile.mkdtemp(prefix="edgeconv_ntff_")
    with hook(neff_dir, [0]):
        results = bass2jax.run_bass_via_pjrt(nc, in_maps, n_cores=CORES)
    exec_ns = None
    try:
        import glob as _glob
        import gauge.profiler
        from concourse._compat import FishPath
        if _glob.glob(os.path.join(neff_dir, "*_body*.ntff")):
            profile = gauge.profiler.Profile(
                profile_path=FishPath(neff_dir), kernel_dev_mode=True,
                profile_on_exit=False, bass_kernel=nc.m,
                offline_processing=True, fname="*_body*")
            pr = profile.to_perfetto(model_index=(0,))
            if pr:
                exec_ns = pr[0].exec_time_ns
                LAST.setdefault("trace_paths", []).append(pr[0].trace_path)
    except Exception as e:  # profiling must never break the kernel
        LAST.setdefault("trace_errors", []).append(repr(e))
    LAST.setdefault("neff_dirs", []).append(neff_dir)
    LAST.setdefault("exec_ns", []).append(exec_ns)
    return results


def kernel(**inputs):
    x = np.asarray(inputs["x"], np.float32)
    edge_index = np.asarray(inputs["edge_index"])
    src = np.asarray(edge_index[0], np.int64)
    dst = np.asarray(edge_index[1], np.int64)

    sched_arr, block_order, order_e, bounds = prep_layout(dst)
    sched = [int(v) for v in sched_arr]
    per_core = prep_edges(src, dst, sched, block_order, order_e, bounds)

    x_pad = np.zeros((NP, C), np.float32)
    x_pad[:N_NODES] = x
    x_bf = x_pad.astype(BF16_NP)

    w1 = fold_weights(np.asarray(inputs["w1a"]), np.asarray(inputs["b1a"]),
                      np.asarray(inputs["g1"]), np.asarray(inputs["be1"]),
                      np.asarray(inputs["rm1"]), np.asarray(inputs["rv1"]),
                      np.asarray(inputs["w1b"]), np.asarray(inputs["b1b"]),
                      BN_EPS)
    w2 = fold_weights(np.asarray(inputs["w2a"]), np.asarray(inputs["b2a"]),
                      np.asarray(inputs["g2"]), np.asarray(inputs["be2"]),
                      np.asarray(inputs["rm2"]), np.asarray(inputs["rv2"]),
                      np.asarray(inputs["w2b"]), np.asarray(inputs["b2b"]),
                      BN_EPS)

    nc1 = build_layer(sched, apply_norm=True)
    r1 = _run(nc1, _layer_inputs(gather_features(x_bf, per_core, block_order),
                                 per_core, w1, max(sched)))
    h = assemble_output(r1, block_order)
    h_bf = h.astype(BF16_NP)

    nc2 = build_layer(sched, apply_norm=False)
    r2 = _run(nc2, _layer_inputs(gather_features(h_bf, per_core, block_order),
                                 per_core, w2, max(sched)))
    out = assemble_output(r2, block_order)

    return np.ascontiguousarray(out[:N_NODES]).astype(np.float32)


# revision 92
# speedup vs baseline: 1.0917x; 1.0022x over previous
"""EdgeConv 2-layer encoder for Trainium2 (Bass/Tile), edge-direct scheme.

Math (one EdgeConv layer, PyG semantics, aggr='add' over dst):
  msg_e = relu(x[dst_e] @ Wu + x[src_e] @ Wv + ba)   Wu=(A_i-A_j).T, Wv=A_j.T
  agg[n] = sum_{e: dst_e = n} msg_e                  (scatter-add)
  conv[n] = agg[n] @ Wb2 + deg[n] * c0               (BN+linear folded)
  layer1: h = l2norm(relu(conv)); layer2: out = conv

Sharding: edges partitioned by dst across 8 cores; each core owns 49
contiguous 128-node blocks. Within a core, blocks are assigned to
program slots sorted by chunk count so the shared SPMD schedule
(max over cores per slot) wastes only a few % padding.

Host pre-stages per core, in slot order (static graph => static layout):
  xsT  [128, TC*128] bf16  x^T columns gathered by edge src
  sTg  [128, TC*128] fp8   one-hot gather  S^T[m, e] = (dst_loc[e] == m)
  s_in [128, TC*128] fp8   one-hot scatter S[e, m]   (same matrix, transposed)
  xt_own [128, 49*128] bf16  own x^T (slot order) for the node phase
Device, node phase: u'[m,c] = x_own@Wu + ba per block -> persistent SBUF.
Device, per 128-edge chunk (dst confined to one 128-node block):
  msg_psum[e,c] = S^T_ch^T @ u'_blk + xsT_ch^T @ Wv    PE (2 mm, psum accum)
  relu per 4-chunk group: psum -> t bf16               ACT
  aggT[c,m] += t^T @ S_ch  (accumulate over chunks)    PE
Block epilogue ([m, c2] orientation, nodes on partitions):
  conv = aggT^T @ Wb2 + deg x c0; layer1 adds relu + per-partition l2norm.
Layer outputs return to host; host re-gathers for layer 2.
"""

import sys

sys.path.insert(0, "/opt/trn_rl_repo")

import numpy as np

from concourse import bacc, bass, mybir, tile

F32 = mybir.dt.float32
BF16 = mybir.dt.bfloat16
FP8 = mybir.dt.float8e4
BF16_NP = mybir.dt.np(BF16)
FP8_NP = mybir.dt.np(FP8)

C = 128
GRP = 4                   # chunks per relu group (one PSUM bank)
CORES = 8
BPC = 49                  # blocks per core
NPC = BPC * C             # nodes per core 6272
NBT = CORES * BPC
NP = NBT * C              # padded node count 50176
N_NODES = 50000
BN_EPS = 1e-5

LAST = {}                 # timing/info stash for test harness


def build_layer(sched: list[int], apply_norm: bool):
    """One EdgeConv layer program (SPMD, same program all cores).
    sched[j] = chunk count of slot j (shared across cores)."""
    TC = sum(sched)
    maxw = max(sched)
    nc = bacc.Bacc("TRN2", num_swdge_queues=4)

    xsT = nc.declare_dram_parameter("xsT", [C, TC * C], BF16, isOutput=False)
    sTg = nc.declare_dram_parameter("sTg", [C, TC * C], FP8, isOutput=False)
    dstc = nc.declare_dram_parameter("dstc", [C, TC], BF16, isOutput=False)
    ir4 = nc.declare_dram_parameter("ir4", [C, maxw, C], BF16, isOutput=False)
    xt_own = nc.declare_dram_parameter("xt_own", [C, BPC * C], BF16,
                                       isOutput=False)
    wv = nc.declare_dram_parameter("wv", [C, C], BF16, isOutput=False)
    wu = nc.declare_dram_parameter("wu", [C, C], BF16, isOutput=False)
    ba = nc.declare_dram_parameter("ba", [1, C], BF16, isOutput=False)
    ones_r = nc.declare_dram_parameter("ones_r", [1, C], BF16, isOutput=False)
    wb2 = nc.declare_dram_parameter("wb2", [C, C], BF16, isOutput=False)
    c0 = nc.declare_dram_parameter("c0", [1, C], BF16, isOutput=False)
    deg = nc.declare_dram_parameter("deg", [1, BPC * C], BF16, isOutput=False)
    out_t = nc.declare_dram_parameter("out_t", [C, BPC * C], BF16,
                                      isOutput=True)

    with tile.TileContext(nc) as tc:
        with (
            tc.tile_pool(name="constp", bufs=1) as constp,
            tc.tile_pool(name="persist", bufs=1) as persist,
            tc.tile_pool(name="blkin", bufs=4) as blkin,
            tc.tile_pool(name="tpool", bufs=4) as tpool,
            tc.tile_pool(name="sgp", bufs=4) as sgp,
            tc.tile_pool(name="outio", bufs=3) as outio,
            tc.tile_pool(name="msgp", bufs=5, space="PSUM") as msgp,
            tc.tile_pool(name="aggp", bufs=2, space="PSUM") as aggp,
            tc.tile_pool(name="convp", bufs=1, space="PSUM") as convp,
        ):
            wu_sb = constp.tile([C, C], BF16, tag="wu")
            nc.sync.dma_start(out=wu_sb[:], in_=wu[:])
            ba_sb = constp.tile([1, C], BF16, tag="ba")
            nc.sync.dma_start(out=ba_sb[:], in_=ba[:])
            onesr_sb = constp.tile([1, C], BF16, tag="onesr")
            nc.sync.dma_start(out=onesr_sb[:], in_=ones_r[:])
            xo_sb = constp.tile([C, BPC * C], BF16, tag="xo")
            nc.sync.dma_start(out=xo_sb[:, : 4 * C], in_=xt_own[:, : 4 * C])
            wv_sb = constp.tile([C, C], BF16, tag="wv")
            nc.sync.dma_start(out=wv_sb[:], in_=wv[:])
            NW = 15
            for w0 in range(4, BPC, NW):
                w1 = min(w0 + NW, BPC)
                nc.sync.dma_start(out=xo_sb[:, w0 * C: w1 * C],
                                  in_=xt_own[:, w0 * C: w1 * C])
            dstc_sb = constp.tile([C, TC], BF16, tag="dstc")
            nc.sync.dma_start(out=dstc_sb[:], in_=dstc[:])
            ir_sb = constp.tile([C, maxw, C], BF16, tag="ir")
            nc.sync.dma_start(out=ir_sb[:], in_=ir4[:])
            wb2_sb = constp.tile([C, C], BF16, tag="wb2")
            nc.sync.dma_start(out=wb2_sb[:], in_=wb2[:])
            c0_sb = constp.tile([1, C], BF16, tag="c0")
            nc.sync.dma_start(out=c0_sb[:], in_=c0[:])
            deg_sb = constp.tile([1, BPC * C], BF16, tag="deg")
            nc.sync.dma_start(out=deg_sb[:], in_=deg[:])

            # node phase: u'[m, c] = x_own_blk @ Wu + ba, kept in SBUF
            u_sb = persist.tile([C, BPC * C], BF16, tag="u")
            for b in range(BPC):
                ups = msgp.tile([C, GRP * C], F32, tag="msg")
                nc.tensor.matmul(ups[:, :C],
                                 lhsT=xo_sb[:, b * C: (b + 1) * C],
                                 rhs=wu_sb[:], start=True, stop=False)
                nc.tensor.matmul(ups[:, :C], lhsT=onesr_sb[:], rhs=ba_sb[:],
                                 start=False, stop=True)
                nc.vector.tensor_copy(out=u_sb[:, b * C: (b + 1) * C],
                                      in_=ups[:, :C])

            # flat group schedule for 1-group software pipelining of
            # the scatter (PE runs next group's msg mms during relu)
            slot_off = [0]
            for v in sched:
                slot_off.append(slot_off[-1] + v)
            groups = []
            for b in range(BPC):
                nch = sched[b]
                for g0 in range(0, nch, GRP):
                    groups.append((b, g0, min(GRP, nch - g0)))

            blk_tiles = {}
            agg_tiles = {}

            def load_block(b):
                nch, off = sched[b], slot_off[b]
                xs_sb = blkin.tile([C, maxw * C], BF16, tag="xs")
                nc.sync.dma_start(out=xs_sb[:, : nch * C],
                                  in_=xsT[:, off * C: (off + nch) * C])
                st_sb = blkin.tile([C, maxw * C], FP8, tag="st")
                nc.sync.dma_start(out=st_sb[:, : nch * C],
                                  in_=sTg[:, off * C: (off + nch) * C])
                s_blk = sgp.tile([C, maxw, C], BF16, tag="sg",
                                 name=f"s_blk_{b}")
                nc.vector.tensor_tensor(
                    out=s_blk[:, :nch, :],
                    in0=ir_sb[:, :nch, :],
                    in1=dstc_sb[:, off: off + nch]
                        .to_broadcast([C, nch, C]),
                    op=mybir.AluOpType.is_equal)
                blk_tiles[b] = (xs_sb, st_sb, s_blk)

            def emit_msg(item):
                b, g0, gw = item
                xs_sb, st_sb, _ = blk_tiles[b]
                msg = msgp.tile([C, GRP * C], F32, tag="msg")
                for j in range(gw):
                    ch = g0 + j
                    sl = msg[:, j * C: (j + 1) * C]
                    nc.tensor.matmul(sl,
                                     lhsT=st_sb[:, ch * C: (ch + 1) * C],
                                     rhs=u_sb[:, b * C: (b + 1) * C],
                                     start=(j == 0), stop=False)
                    nc.tensor.matmul(sl,
                                     lhsT=xs_sb[:, ch * C: (ch + 1) * C],
                                     rhs=wv_sb[:], start=False,
                                     stop=(j == gw - 1))
                t_g = tpool.tile([C, GRP * C], BF16, tag="t")
                emit_msg.ctr += 1
                if not apply_norm and emit_msg.ctr % 5 == 0:
                    nc.vector.tensor_scalar(
                        out=t_g[:, : gw * C], in0=msg[:, : gw * C],
                        scalar1=0.0, scalar2=None,
                        op0=mybir.AluOpType.max)
                else:
                    nc.scalar.activation(
                        out=t_g[:, : gw * C], in_=msg[:, : gw * C],
                        func=mybir.ActivationFunctionType.Relu)
                return t_g

            emit_msg.ctr = 0

            def emit_scatter(item, t_g):
                b, g0, gw = item
                nch = sched[b]
                s_blk = blk_tiles[b][2]
                if b not in agg_tiles:
                    agg_tiles[b] = aggp.tile([C, C], F32, tag="agg",
                                             name=f"aggT_{b}")
                aggT = agg_tiles[b]
                for j in range(gw):
                    ch = g0 + j
                    nc.tensor.matmul(aggT[:],
                                     lhsT=t_g[:, j * C: (j + 1) * C],
                                     rhs=s_blk[:, ch, :],
                                     start=(ch == 0), stop=(ch == nch - 1))

            def emit_cast(b):
                agg_sb = outio.tile([C, C], BF16, tag="aggsb")
                nc.vector.tensor_copy(out=agg_sb[:], in_=agg_tiles.pop(b)[:])
                del blk_tiles[b]
                return agg_sb

            def emit_epilogue(b, agg_sb):
                # conv in [m, c2] orientation: nodes on partitions
                cps = convp.tile([C, C], F32, tag="conv")
                nc.tensor.matmul(cps[:], lhsT=agg_sb[:], rhs=wb2_sb[:],
                                 start=True, stop=False)
                nc.tensor.matmul(cps[:],
                                 lhsT=deg_sb[0:1, b * C: (b + 1) * C],
                                 rhs=c0_sb[:], start=False, stop=True)
                o_sb = outio.tile([C, C], BF16, tag="o")
                if apply_norm:
                    h_sb = outio.tile([C, C], BF16, tag="h")
                    nc.scalar.activation(out=h_sb[:], in_=cps[:],
                                         func=mybir.ActivationFunctionType.Relu)
                    sq_sb = outio.tile([C, C], BF16, tag="sq")
                    nc.vector.tensor_tensor(out=sq_sb[:], in0=h_sb[:],
                                            in1=h_sb[:],
                                            op=mybir.AluOpType.mult)
                    ssq = outio.tile([C, 1], F32, tag="ssq")
                    nc.vector.tensor_reduce(out=ssq[:], in_=sq_sb[:],
                                            axis=mybir.AxisListType.X,
                                            op=mybir.AluOpType.add)
                    nrm = outio.tile([C, 1], F32, tag="nrm")
                    nc.scalar.activation(out=nrm[:], in_=ssq[:],
                                         func=mybir.ActivationFunctionType.Sqrt)
                    nc.vector.tensor_scalar(out=nrm[:], in0=nrm[:],
                                            scalar1=1e-12, scalar2=None,
                                            op0=mybir.AluOpType.max)
                    nc.vector.reciprocal(out=nrm[:], in_=nrm[:])
                    nc.vector.tensor_scalar(out=o_sb[:], in0=h_sb[:],
                                            scalar1=nrm[:], scalar2=None,
                                            op0=mybir.AluOpType.mult)
                else:
                    nc.scalar.activation(out=o_sb[:], in_=cps[:],
                                         func=mybir.ActivationFunctionType.Copy)
                nc.sync.dma_start(out=out_t[:, b * C: (b + 1) * C], in_=o_sb[:])

            from collections import deque
            load_block(0)
            load_block(1)
            pending = deque()  # (item, t_g, s_g) awaiting scatter
            pend_cast = None   # block id awaiting cast
            pend_ep = None     # (b, agg_sb) awaiting conv/norm
            for item in groups:
                b, g0, gw = item
                if g0 == 0 and b + 2 < BPC:
                    load_block(b + 2)
                t_g = emit_msg(item)
                if pend_ep is not None:
                    emit_epilogue(*pend_ep)
                    pend_ep = None
                if len(pending) >= 2:
                    pit, pt = pending.popleft()
                    emit_scatter(pit, pt)
                    pb, pg0, pgw = pit
                    if pg0 + pgw >= sched[pb]:      # block pb finished
                        pend_cast = pb
                if pend_cast is not None:
                    pend_ep = (pend_cast, emit_cast(pend_cast))
                    pend_cast = None
                pending.append((item, t_g))
            if pend_ep is not None:
                emit_epilogue(*pend_ep)
                pend_ep = None
            while pending:
                pit, pt = pending.popleft()
                emit_scatter(pit, pt)
                pb, pg0, pgw = pit
                if pg0 + pgw >= sched[pb]:
                    emit_epilogue(pb, emit_cast(pb))

    nc.compile()
    return nc


# ---------------- host-side data prep ----------------


def prep_layout(dst):
    """Slot-sorted schedule: per core, blocks ordered by descending chunk
    count; sched[j] = max over cores of j-th largest."""
    order_e = np.argsort(dst, kind="stable")
    dst_s = dst[order_e]
    bounds = np.searchsorted(dst_s, np.arange(NBT + 1) * C, side="left")
    counts = (bounds[1:] - bounds[:-1]).reshape(CORES, BPC)
    chunks = -(-counts // C)          # ceil
    block_order = np.argsort(-chunks, axis=1, kind="stable")
    sorted_chunks = -np.sort(-chunks, axis=1)
    sched = sorted_chunks.max(axis=0)
    return sched, block_order, order_e, bounds


def prep_edges(src, dst, sched, block_order, order_e, bounds):
    """Per-core slot-ordered edge arrays, one-hot tables, slot deg."""
    TC = sum(int(x) for x in sched)
    slot_off = np.concatenate([[0], np.cumsum(sched)])
    deg_full = np.bincount(dst, minlength=NP)
    per_core = []
    for k in range(CORES):
        src_perm = np.zeros(TC * C, np.int64)
        dstc_v = np.full(TC * C, 200.0, np.float64)
        deg_slot = np.zeros(BPC * C, np.float64)
        for j in range(BPC):
            blk = int(block_order[k, j])
            g = k * BPC + blk
            e_idx = order_e[bounds[g]: bounds[g + 1]]
            n = len(e_idx)
            base = int(slot_off[j]) * C
            src_perm[base: base + n] = src[e_idx]
            dstc_v[base: base + n] = dst[e_idx] % C
            deg_slot[j * C: (j + 1) * C] = deg_full[k * NPC + blk * C:
                                                    k * NPC + (blk + 1) * C]
        d = dstc_v.reshape(TC, C)
        mr = np.arange(C, dtype=np.float64)
        # gather one-hot sT[m, slot*C + e]; scatter one-hot built on-device
        st_oh = np.ascontiguousarray(
            (d[None, :, :] == mr[:, None, None]).astype(FP8_NP)
        ).reshape(C, TC * C)
        per_core.append({
            "src_perm": src_perm,
            "st_oh": st_oh,
            "dstc": np.ascontiguousarray(d.T.astype(BF16_NP)),
            "deg": np.ascontiguousarray(
                deg_slot.reshape(1, BPC * C).astype(BF16_NP)),
        })
    return per_core


def gather_features(x_bf, per_core, block_order):
    """x_bf [NP, C] bf16 -> per-core (xsT [C, TC*C], xt_own [C, BPC*C])."""
    outs = []
    for k in range(CORES):
        xs = np.ascontiguousarray(x_bf[per_core[k]["src_perm"]].T)
        own = np.empty((BPC, C, C), BF16_NP)
        for j in range(BPC):
            blk = int(block_order[k, j])
            own[j] = x_bf[k * NPC + blk * C: k * NPC + (blk + 1) * C].T
        xo = np.ascontiguousarray(own.transpose(1, 0, 2).reshape(C, BPC * C))
        outs.append((xs, xo))
    return outs


def fold_weights(wa, ba_, g, be, rm, rv, wb, bb, bn_eps=1e-5):
    wa = wa.astype(np.float64)
    A_i, A_j = wa[:, :C], wa[:, C:]
    s = g.astype(np.float64) / np.sqrt(rv.astype(np.float64) + bn_eps)
    wb64 = wb.astype(np.float64)
    wu_m = (A_i - A_j).T
    wv_m = A_j.T
    wb2 = s[:, None] * wb64.T
    c0 = bb.astype(np.float64) + (be.astype(np.float64)
                                  - rm.astype(np.float64) * s) @ wb64.T
    return (wu_m.astype(BF16_NP), wv_m.astype(BF16_NP),
            ba_.astype(BF16_NP).reshape(1, C),
            wb2.astype(BF16_NP), c0.astype(BF16_NP).reshape(1, C))


def _layer_inputs(feat, per_core, wset, maxw):
    wu_m, wv_m, ba_f, wb2, c0 = wset
    onesr = np.ones((1, C), BF16_NP)
    ir = np.tile(np.arange(C, dtype=np.float64), (C, maxw, 1)).astype(BF16_NP)
    in_maps = []
    for k in range(CORES):
        xs, xo = feat[k]
        in_maps.append({
            "xsT": xs, "xt_own": xo,
            "sTg": per_core[k]["st_oh"], "dstc": per_core[k]["dstc"],
            "ir4": ir, "deg": per_core[k]["deg"],
            "wv": wv_m, "wu": wu_m, "ba": ba_f, "wb2": wb2, "c0": c0,
            "ones_r": onesr,
        })
    return in_maps


def assemble_output(results, block_order):
    """Per-core out_t [128(m), BPC*C] slot tiles [m, c2] -> global [NP, C]."""
    h = np.zeros((NP, C), np.float32)
    for k in range(CORES):
        o = np.asarray(results[k]["out_t"]).astype(np.float32)
        o = o.reshape(C, BPC, C).transpose(1, 0, 2)   # [slot, m, c]
        for j in range(BPC):
            blk = int(block_order[k, j])
            h[k * NPC + blk * C: k * NPC + (blk + 1) * C] = o[j]
    return h


# ---------------- device run plumbing ----------------

import os

_NTFF_HOOK = None


def _get_ntff_hook():
    global _NTFF_HOOK
    if _NTFF_HOOK is None:
        sys.path.insert(0, "/root/.axon_site")
        from trn_agent_boot.trn_boot import _ntff_profile_via_ctypes
        _NTFF_HOOK = _ntff_profile_via_ctypes("/opt/axon/libaxon_pjrt.so")
    return _NTFF_HOOK


def _run(nc, in_maps):
    import tempfile
    from concourse import bass2jax
    trace = bool(int(os.environ.get("EDGECONV_TRACE", "0")))
    hook = _get_ntff_hook() if trace else None
    if hook is None:
        results = bass2jax.run_bass_via_pjrt(nc, in_maps, n_cores=CORES)
        LAST.setdefault("exec_ns", []).append(None)
        return results
    neff_dir = tempfile.mkdtemp(prefix="edgeconv_ntff_")
    with hook(neff_dir, [0]):
        results = bass2jax.run_bass_via_pjrt(nc, in_maps, n_cores=CORES)
    exec_ns = None
    try:
        import glob as _glob
        import gauge.profiler
        from concourse._compat import FishPath
        if _glob.glob(os.path.join(neff_dir, "*_body*.ntff")):
            profile = gauge.profiler.Profile(
                profile_path=FishPath(neff_dir), kernel_dev_mode=True,
                profile_on_exit=False, bass_kernel=nc.m,
                offline_processing=True, fname="*_body*")
            pr = profile.to_perfetto(model_index=(0,))
            if pr:
                exec_ns = pr[0].exec_time_ns
                LAST.setdefault("trace_paths", []).append(pr[0].trace_path)
    except Exception as e:  # profiling must never break the kernel
        LAST.setdefault("trace_errors", []).append(repr(e))
    LAST.setdefault("neff_dirs", []).append(neff_dir)
    LAST.setdefault("exec_ns", []).append(exec_ns)
    return results


def kernel(**inputs):
    x = np.asarray(inputs["x"], np.float32)
    edge_index = np.asarray(inputs["edge_index"])
    src = np.asarray(edge_index[0], np.int64)
    dst = np.asarray(edge_index[1], np.int64)

    sched_arr, block_order, order_e, bounds = prep_layout(dst)
    sched = [int(v) for v in sched_arr]
    per_core = prep_edges(src, dst, sched, block_order, order_e, bounds)

    x_pad = np.zeros((NP, C), np.float32)
    x_pad[:N_NODES] = x
    x_bf = x_pad.astype(BF16_NP)

    w1 = fold_weights(np.asarray(inputs["w1a"]), np.asarray(inputs["b1a"]),
                      np.asarray(inputs["g1"]), np.asarray(inputs["be1"]),
                      np.asarray(inputs["rm1"]), np.asarray(inputs["rv1"]),
                      np.asarray(inputs["w1b"]), np.asarray(inputs["b1b"]),
                      BN_EPS)
    w2 = fold_weights(np.asarray(inputs["w2a"]), np.asarray(inputs["b2a"]),
                      np.asarray(inputs["g2"]), np.asarray(inputs["be2"]),
                      np.asarray(inputs["rm2"]), np.asarray(inputs["rv2"]),
                      np.asarray(inputs["w2b"]), np.asarray(inputs["b2b"]),
                      BN_EPS)

    nc1 = build_layer(sched, apply_norm=True)
    r1 = _run(nc1, _layer_inputs(gather_features(x_bf, per_core, block_order),
                                 per_core, w1, max(sched)))
    h = assemble_output(r1, block_order)
    h_bf = h.astype(BF16_NP)

    nc2 = build_layer(sched, apply_norm=False)
    r2 = _run(nc2, _layer_inputs(gather_features(h_bf, per_core, block_order),
                                 per_core, w2, max(sched)))
    out = assemble_output(r2, block_order)

    return np.ascontiguousarray(out[:N_NODES]).astype(np.float32)
